# revision 13
# baseline (speedup 1.0000x reference)
"""Cross-attention fusion kernel for Trainium2 (8 NeuronCores).

Reference computation (per sample b):
    q = Wq @ xs + bq            xs = x_s2[b] as [256, 4096]
    k = Wk @ xd + bk            xd = x_dem[b] as [64, 4096]
    v = Wv @ xd + bv
    attn = softmax_j(k^T q * c)             c = 256 ** -0.5
    out = v @ attn + x_s2[b]                out[ch, j] = sum_i v[ch, i] attn[i, j]

Device-side restructure (mathematically identical):
  - logits z = k^T q * c = xda^T mx with xda = [xd; 1] and
    mx = ([Wk^T; bk] @ (Wq * c)) @ xs precomputed on the host ([65, n]).
    The contraction is rank-65, so phase D runs one K=65 matmul per tile
    instead of a K=256 pair; the bk-induced per-query bias rides the ones
    row exactly and bq cancels in softmax_j -> dropped.
  - e-matrix dual path, assigned per i-block (so each softmax row has one
    consistent scale that its own row sum cancels):
      ACT blocks: exp(z - ln4) via the activation LUT, fp8 out, row sums
        via the free accumulator.
      DVE blocks: Schraudolph bitcast exp - one tensor_scalar computes
        int32(A*z + B) (the affine + f32->int32 convert fused), a second
        reads the bits as f32 and casts to fp8 with a fused row-sum
        accumulate. B encodes the same -ln4 shift, so both paths match.
  - softmax denominators folded into v columns (fp8 vts = v * r * ALPHA_V).
  - out-matmul in fp8 DoubleRow (K=256/instr), PSUM evicted as bf16 by
    ACT/DVE alternately, DMA'd per 512-column group.
  - Dummy matmuls (overwritten by the next start=True real matmul) pad the
    PE's duty cycle during the drain-bound phase D so the HAM clock gate
    holds 2.4 GHz; without them PE duty ~45% lets the clock fall to 1.2 GHz
    (the activity monitor needs a ~3.4us window of near-continuous busy).

Sharding: 8 cores = 4 samples x 2 halves of the key-pixel axis i. Each core
emits a partial out [256, 4096]; the host sums the halves and adds the
residual. No collectives.
"""

import numpy as np
import ml_dtypes

import concourse.bass as bass
import concourse.mybir as mybir
import concourse.tile as tile
from concourse import bacc
from concourse.bass_utils import run_bass_kernel_spmd

P = 128
CH = 256          # out_ch == s2_ch
DEM = 64          # dem_ch
DEM1 = DEM + 1    # + ones row (bk/bv fold)
N = 4096          # pixels per sample (j axis)
NI = 2048         # key pixels per core (i axis, half of N)
NIB = NI // P     # 16 i-blocks per core
NPAIR = NIB // 2
KO = CH // P      # 2 partition chunks of the 256-channel out axis
NCORES = 8

F32 = mybir.dt.float32
BF16 = mybir.dt.bfloat16
FP8 = mybir.dt.float8e4
I8 = mybir.dt.int8
NP_BF16 = ml_dtypes.bfloat16

# fp8 scale plan for the out-matmul (phase E): e is stored as exp(z - ln4)
# (max ~166, inside e4m3 range) and vts as v * r * ALPHA_V (O(1) values).
ALPHA_V = 8192.0
E_BIAS = -1.3862943611198906  # -ln(4)

# fp8-bit-space exp for the DVE blocks: PSUM holds b = S8*z + B8 (mx is
# pre-scaled on the host; B8 rides the ones row). int8(clamp(b, 0, 126))'s
# bit pattern IS the fp8e4m3 encoding of ~K*exp(z) (log-linear code trick,
# rms ~3%). Row sums are measured on the STORED fp8 values, so the bit-space
# scale K cancels per row - no calibration needed.
S8 = 8.0 / np.log(2.0)
B8 = 43.75                        # bf16-exact, rms-optimal code offset
LN2_8 = float(np.log(2.0) / 8.0)  # ACT scale back to z-domain
ACT_BIAS = float(-B8 * np.log(2.0) / 8.0 - np.log(4.0))

# i-blocks whose e-tiles take the DVE bit-space path (5 of 16, spread out;
# the last blocks stay on ACT so the phase-D drain tail is short).
DVE_IBS = frozenset((3, 5, 7, 9, 11))


def build_bass():
    nc = bacc.Bacc(None, target_bir_lowering=False)

    xda_d = nc.dram_tensor("xda", [P, NI], BF16, kind="ExternalInput")
    mx_d = nc.dram_tensor("mx", [P, N], BF16, kind="ExternalInput")
    wv_d = nc.dram_tensor("wv", [DEM1, CH], BF16, kind="ExternalInput")
    out_d = nc.dram_tensor("out", [CH, N], BF16, kind="ExternalOutput")

    out_v = out_d.ap().rearrange("(m p) j -> p m j", p=P)

    with tile.TileContext(nc) as tc:
        with (
            tc.tile_pool(name="consts", bufs=1) as consts,
            tc.tile_pool(name="bigs", bufs=1) as bigs,
            tc.tile_pool(name="small", bufs=1) as small,
            tc.tile_pool(name="sstage", bufs=2) as sstage,
            tc.tile_pool(name="stage", bufs=2) as stage,
        ):
            xda_sb = consts.tile([P, NI], BF16)
            nc.sync.dma_start(out=xda_sb, in_=xda_d.ap())
            wv_sb = consts.tile([DEM1, CH], BF16)
            nc.sync.dma_start(out=wv_sb, in_=wv_d.ap())
            mx_sb = consts.tile([P, N], BF16)
            for jh in range(2):
                nc.sync.dma_start(
                    out=mx_sb[:, jh * 2048:(jh + 1) * 2048],
                    in_=mx_d.ap()[:, jh * 2048:(jh + 1) * 2048],
                )

            vt_sb = bigs.tile([P, NIB, CH], BF16)  # v^T[i, ch], i on partitions
            e_sb = bigs.tile([P, NIB, N], FP8)     # exp(z - ln4) [i, j]
            vts_sb = bigs.tile([P, NIB, CH], FP8)  # v^T * r * ALPHA_V

            r_sb = small.tile([P, NIB], F32)
            sums_sb = small.tile([P, NIB, 2], F32)
            sums2_sb = small.tile([P, 2, 4], F32)
            ebias_sb = small.tile([P, 1], F32)
            nc.vector.memset(ebias_sb, ACT_BIAS)
            warm_sb = small.tile([P, 512], BF16)
            nc.vector.memset(warm_sb, 0.0)

            def emit_b_group(mm_pool, g):
                ps = mm_pool.tile([P, 2048], F32, tag="ps", name=f"bps_{g}")
                for q in range(4):
                    ib = g * 4 + q
                    nc.tensor.matmul(
                        ps[:, q * 512:q * 512 + CH],
                        lhsT=xda_sb[:DEM1, ib * P:(ib + 1) * P],
                        rhs=wv_sb,
                        start=True, stop=True,
                    )
                ps_v = ps.rearrange("p (q s) -> p q s", s=512)[:, :, :CH]
                if g % 2 == 0:
                    nc.vector.tensor_copy(
                        out=vt_sb[:, g * 4:(g + 1) * 4, :], in_=ps_v
                    )
                else:
                    nc.scalar.copy(
                        out=vt_sb[:, g * 4:(g + 1) * 4, :], in_=ps_v
                    )

            def emit_evict(ib, jp, pp, jsl):
                if ib in DVE_IBS:
                    # clamp to 0x77: TRN fp8e4 is IEEE-style, exp 1111 is
                    # Inf/NaN and max normal is 240 (bits 119)
                    nc.vector.tensor_scalar(
                        out=e_sb[:, ib, jsl].bitcast(I8),
                        in0=pp,
                        scalar1=0.0, scalar2=119.0,
                        op0=mybir.AluOpType.max,
                        op1=mybir.AluOpType.min,
                    )
                    nc.vector.tensor_reduce(
                        out=sums_sb[:, ib, jp:jp + 1],
                        in_=e_sb[:, ib, jsl],
                        axis=mybir.AxisListType.X,
                        op=mybir.AluOpType.add,
                    )
                else:
                    nc.scalar.activation(
                        out=e_sb[:, ib, jsl],
                        in_=pp,
                        func=mybir.ActivationFunctionType.Exp,
                        scale=LN2_8,
                        bias=ebias_sb,
                        accum_out=sums_sb[:, ib, jp:jp + 1],
                    )

            def emit_ib_tail(ib):
                nc.vector.reduce_sum(
                    out=r_sb[:, ib:ib + 1],
                    in_=sums_sb[:, ib, :],
                    axis=mybir.AxisListType.X,
                )
                nc.vector.reciprocal(
                    out=r_sb[:, ib:ib + 1], in_=r_sb[:, ib:ib + 1]
                )
                nc.gpsimd.tensor_scalar(
                    out=vts_sb[:, ib, :],
                    in0=vt_sb[:, ib, :],
                    scalar1=r_sb[:, ib:ib + 1],
                    scalar2=ALPHA_V,
                    op0=mybir.AluOpType.mult,
                    op1=mybir.AluOpType.mult,
                )

            with tc.tile_pool(name="mm_psum", bufs=2, space="PSUM") as mm_psum:
                # HAM warm-up: back-to-back K=128 matmuls until the first mx
                # half lands; the clock flips after one full ~3.4us window.
                wp = mm_psum.tile([P, 2048], F32, tag="ps")
                for w in range(8):
                    nc.tensor.matmul(
                        wp[:, (w % 4) * 512:(w % 4) * 512 + 512],
                        lhsT=warm_sb[:, :P],
                        rhs=warm_sb,
                        start=True, stop=True,
                    )

                # ---- Phase D (i-blocks 0..13), phase B interleaved ----
                # B's K=65 matmuls are invisible to the HAM activity monitor,
                # so they ride inside the K=128-rich phase D instead of
                # forming their own (clock-cooling) phase.
                for ib in range(NIB - 2):
                    for jp in range(2):
                        pp = mm_psum.tile([P, 2048], F32, tag="ps")
                        # dummy matmul pads PE duty; garbage overwritten by
                        # the start=True matmuls below.
                        nc.tensor.matmul(
                            pp[:, 0:512],
                            lhsT=warm_sb[:, :P], rhs=warm_sb,
                            start=True, stop=True,
                        )
                        for jj in range(4):
                            j0 = jp * 2048 + jj * 512
                            nc.tensor.matmul(
                                pp[:, jj * 512:(jj + 1) * 512],
                                lhsT=xda_sb[:, ib * P:(ib + 1) * P],
                                rhs=mx_sb[:, j0:j0 + 512],
                                start=True, stop=True,
                            )
                        emit_evict(ib, jp, pp, slice(jp * 2048, (jp + 1) * 2048))
                    if ib < 4:
                        emit_b_group(mm_psum, ib)
                    emit_ib_tail(ib)

            # ---- last two i-blocks on a 4-bank pool + phase E ----
            # mm2 (4 banks) + ep (4 banks) coexist, so phase E's early groups
            # run while i-blocks 14/15 still compute and drain.
            with (
                tc.tile_pool(name="mm2", bufs=2, space="PSUM") as mm2,
                tc.tile_pool(name="ep", bufs=2, space="PSUM") as ep,
            ):
                for ib in (NIB - 2, NIB - 1):
                    for jq in range(4):
                        pp = mm2.tile([P, 1024], F32, tag="p2",
                                      name=f"p2_{ib}_{jq}")
                        nc.tensor.matmul(
                            pp[:, 0:512],
                            lhsT=warm_sb[:, :P], rhs=warm_sb,
                            start=True, stop=True,
                        )
                        for jj in range(2):
                            j0 = jq * 1024 + jj * 512
                            nc.tensor.matmul(
                                pp[:, jj * 512:(jj + 1) * 512],
                                lhsT=xda_sb[:, ib * P:(ib + 1) * P],
                                rhs=mx_sb[:, j0:j0 + 512],
                                start=True, stop=True,
                            )
                        jsl = slice(jq * 1024, (jq + 1) * 1024)
                        nc.scalar.activation(
                            out=e_sb[:, ib, jsl],
                            in_=pp,
                            func=mybir.ActivationFunctionType.Exp,
                            scale=LN2_8,
                            bias=ebias_sb,
                            accum_out=sums2_sb[:, ib - (NIB - 2), jq:jq + 1],
                        )
                    nc.vector.reduce_sum(
                        out=r_sb[:, ib:ib + 1],
                        in_=sums2_sb[:, ib - (NIB - 2), :],
                        axis=mybir.AxisListType.X,
                    )
                    nc.vector.reciprocal(
                        out=r_sb[:, ib:ib + 1], in_=r_sb[:, ib:ib + 1]
                    )
                    nc.gpsimd.tensor_scalar(
                        out=vts_sb[:, ib, :],
                        in0=vt_sb[:, ib, :],
                        scalar1=r_sb[:, ib:ib + 1],
                        scalar2=ALPHA_V,
                        op0=mybir.AluOpType.mult,
                        op1=mybir.AluOpType.mult,
                    )

                # ---- Phase E: out[ch, j] = sum_i vts[i, ch] e[i, j] ----
                for jc in range(8):
                    pq = ep.tile([P, KO, 512], F32, tag="po", name=f"po_{jc}")
                    for ibp in range(NPAIR):
                        for m in range(KO):
                            nc.tensor.matmul(
                                pq[:, m, :],
                                lhsT=vts_sb[:, 2 * ibp:2 * ibp + 2,
                                            m * P:(m + 1) * P],
                                rhs=e_sb[:, 2 * ibp:2 * ibp + 2,
                                         jc * 512:(jc + 1) * 512],
                                start=(ibp == 0), stop=(ibp == NPAIR - 1),
                                perf_mode=mybir.MatmulPerfMode.DoubleRow,
                            )
                    sth = stage.tile([P, KO, 512], BF16, tag="st",
                                     name=f"st_{jc}")
                    if jc % 2 == 0:
                        nc.scalar.mul(out=sth, in_=pq, mul=1.0 / ALPHA_V)
                    else:
                        nc.vector.tensor_scalar_mul(
                            out=sth, in0=pq, scalar1=1.0 / ALPHA_V
                        )
                    nc.sync.dma_start(
                        out=out_v[:, :, jc * 512:(jc + 1) * 512],
                        in_=sth,
                    )
    nc.finalize()
    return nc


_NC_CACHE = None


def _get_nc():
    global _NC_CACHE
    if _NC_CACHE is None:
        _NC_CACHE = build_bass()
    return _NC_CACHE


def make_in_maps(x_s2, x_dem, Wq, bq, Wk, bk, Wv, bv):
    scale = np.float32(CH ** -0.5)
    wk_aug = np.concatenate([Wk.T, bk[None, :]], axis=0)                 # [65, 256]
    wm = wk_aug @ (Wq * scale)                                           # [65, 256]
    wv_aug = np.concatenate([Wv.T, bv[None, :]], axis=0).astype(NP_BF16)
    ones = np.ones((1, NI), np.float32)
    B = x_s2.shape[0]
    mxs = []
    for s in range(B):
        xs = x_s2[s].reshape(CH, N)
        mx = wm @ xs                                                     # [65, 4096]
        mxb = np.zeros((P, N), np.float32)
        mxb[:DEM] = S8 * mx[:DEM]
        mxb[DEM] = B8 + S8 * mx[DEM]      # ones row carries B8 + bk-bias
        mxs.append(np.ascontiguousarray(mxb).astype(NP_BF16))
    in_maps = []
    for c in range(NCORES):
        s, h = divmod(c, 2)
        xd = x_dem[s].reshape(DEM, N)[:, h * NI:(h + 1) * NI]
        xda = np.zeros((P, NI), np.float32)
        xda[:DEM] = xd
        xda[DEM] = 1.0
        in_maps.append({"xda": np.ascontiguousarray(xda.astype(NP_BF16)),
                        "mx": mxs[s], "wv": wv_aug})
    return in_maps


def run(inputs, trace=False, trace_cores=None):
    """Run the device kernel; returns (output, BassKernelResults)."""
    x_s2 = np.asarray(inputs["x_s2"], np.float32)
    x_dem = np.asarray(inputs["x_dem"], np.float32)
    args = {k: np.asarray(inputs[k], np.float32)
            for k in ("Wq", "bq", "Wk", "bk", "Wv", "bv")}
    in_maps = make_in_maps(x_s2, x_dem, args["Wq"], args["bq"],
                           args["Wk"], args["bk"], args["Wv"], args["bv"])
    nc = _get_nc()
    res = run_bass_kernel_spmd(nc, in_maps, core_ids=list(range(NCORES)),
                               trace=trace, trace_cores=trace_cores)
    B = x_s2.shape[0]
    out = np.empty_like(x_s2)
    for s in range(B):
        part = (res.results[2 * s]["out"].astype(np.float32)
                + res.results[2 * s + 1]["out"].astype(np.float32))
        out[s] = part.reshape(CH, 64, 64) + x_s2[s]
    return out, res


def kernel(**inputs):
    out, _ = run(inputs, trace=False)
    return out


# revision 16
# speedup vs baseline: 1.0003x; 1.0003x over previous
"""Cross-attention fusion kernel for Trainium2 (8 NeuronCores).

Reference computation (per sample b):
    q = Wq @ xs + bq            xs = x_s2[b] as [256, 4096]
    k = Wk @ xd + bk            xd = x_dem[b] as [64, 4096]
    v = Wv @ xd + bv
    attn = softmax_j(k^T q * c)             c = 256 ** -0.5
    out = v @ attn + x_s2[b]                out[ch, j] = sum_i v[ch, i] attn[i, j]

Device-side restructure (mathematically identical):
  - logits z = k^T q * c = xda^T mx with xda = [xd; 1] and
    mx = ([Wk^T; bk] @ (Wq * c)) @ xs precomputed on the host ([65, n]).
    The contraction is rank-65, so phase D runs one K=65 matmul per tile
    instead of a K=256 pair; the bk-induced per-query bias rides the ones
    row exactly and bq cancels in softmax_j -> dropped.
  - every psum tile's drain is COLUMN-SPLIT across two engines working
    concurrently (the psum pool only holds 2 tiles, so per-tile parallelism
    is the only way ACT and DVE overlap): ACT runs exp(z - ln4) via the
    activation LUT with free row-sum accumulation on the left columns; DVE
    runs Schraudolph bitcast exp (int32(A*z + B) then a bits-as-f32 cast to
    fp8 with fused accumulate) on the right columns. B encodes the same
    -ln4 shift, so the two column ranges share one scale inside each row.
  - softmax denominators folded into v columns (fp8 vts = v * r * ALPHA_V).
  - out-matmul in fp8 DoubleRow (K=256/instr), PSUM evicted as bf16 by
    ACT/DVE alternately, DMA'd per 512-column group.
  - Dummy matmuls (overwritten by the next start=True real matmul) pad the
    PE's duty cycle during the drain-bound phase D so the HAM clock gate
    holds 2.4 GHz; without them PE duty ~45% lets the clock fall to 1.2 GHz
    (the activity monitor needs a ~3.4us window of near-continuous busy).

Sharding: 8 cores = 4 samples x 2 halves of the key-pixel axis i. Each core
emits a partial out [256, 4096]; the host sums the halves and adds the
residual. No collectives.
"""

import numpy as np
import ml_dtypes

import concourse.bass as bass
import concourse.mybir as mybir
import concourse.tile as tile
from concourse import bacc
from concourse.bass_utils import run_bass_kernel_spmd

P = 128
CH = 256          # out_ch == s2_ch
DEM = 64          # dem_ch
DEM1 = DEM + 1    # + ones row (bk/bv fold)
N = 4096          # pixels per sample (j axis)
NI = 2048         # key pixels per core (i axis, half of N)
NIB = NI // P     # 16 i-blocks per core
NPAIR = NIB // 2
KO = CH // P      # 2 partition chunks of the 256-channel out axis
NCORES = 8

F32 = mybir.dt.float32
BF16 = mybir.dt.bfloat16
FP8 = mybir.dt.float8e4
I32 = mybir.dt.int32
NP_BF16 = ml_dtypes.bfloat16

# fp8 scale plan for the out-matmul (phase E): e is stored as exp(z - ln4)
# (max ~166, inside e4m3 range) and vts as v * r * ALPHA_V (O(1) values).
ALPHA_V = 8192.0
E_BIAS = -1.3862943611198906  # -ln(4)

# Schraudolph exp for the DVE-drained columns: exp(z - ln4) ~
# bitcast(int32(A*z + B)). A = 2^23/ln2; B = (127 - 2)*2^23 - C (the -2
# absorbs the -ln4 shift); C = 486500 minimizes RMS relative error (~1.8%,
# max ~3.9%, mean ~0 - scale-consistent with the ACT columns, so ACT and
# DVE columns can mix freely inside one softmax row).
SCHRAU_A = 12102203.161561485
SCHRAU_B = 125.0 * 2.0**23 - 486500.0

# Per-tile drain split: ACT handles cols [0, XSPL), DVE cols [XSPL, 2048).
# Balances (x+352)/1.2+182 on ACT against 2*(2048-x+120)/0.96 on DVE, so
# both engines finish a tile's drain together (no convoy).
XSPL = 1408


def build_bass():
    nc = bacc.Bacc(None, target_bir_lowering=False)

    xda_d = nc.dram_tensor("xda", [P, NI], BF16, kind="ExternalInput")
    mx_d = nc.dram_tensor("mx", [P, N], BF16, kind="ExternalInput")
    wv_d = nc.dram_tensor("wv", [DEM1, CH], BF16, kind="ExternalInput")
    out_d = nc.dram_tensor("out", [CH, N], BF16, kind="ExternalOutput")

    out_v = out_d.ap().rearrange("(m p) j -> p m j", p=P)

    with tile.TileContext(nc) as tc:
        with (
            tc.tile_pool(name="consts", bufs=1) as consts,
            tc.tile_pool(name="bigs", bufs=1) as bigs,
            tc.tile_pool(name="small", bufs=1) as small,
            tc.tile_pool(name="sstage", bufs=2) as sstage,
            tc.tile_pool(name="stage", bufs=2) as stage,
        ):
            xda_sb = consts.tile([P, NI], BF16)
            nc.sync.dma_start(out=xda_sb, in_=xda_d.ap())
            wv_sb = consts.tile([DEM1, CH], BF16)
            nc.sync.dma_start(out=wv_sb, in_=wv_d.ap())
            mx_sb = consts.tile([P, N], BF16)
            for jh in range(2):
                nc.sync.dma_start(
                    out=mx_sb[:, jh * 2048:(jh + 1) * 2048],
                    in_=mx_d.ap()[:, jh * 2048:(jh + 1) * 2048],
                )

            vt_sb = bigs.tile([P, NIB, CH], BF16)  # v^T[i, ch], i on partitions
            e_sb = bigs.tile([P, NIB, N], FP8)     # exp(z - ln4) [i, j]
            vts_sb = bigs.tile([P, NIB, CH], FP8)  # v^T * r * ALPHA_V

            r_sb = small.tile([P, NIB], F32)
            sums_sb = small.tile([P, NIB, 4], F32)
            sums2_sb = small.tile([P, 2, 8], F32)
            ebias_sb = small.tile([P, 1], F32)
            nc.vector.memset(ebias_sb, E_BIAS)
            warm_sb = small.tile([P, 512], BF16)
            nc.vector.memset(warm_sb, 0.0)

            def emit_b_group(mm_pool, g):
                ps = mm_pool.tile([P, 2048], F32, tag="ps", name=f"bps_{g}")
                for q in range(4):
                    ib = g * 4 + q
                    nc.tensor.matmul(
                        ps[:, q * 512:q * 512 + CH],
                        lhsT=xda_sb[:DEM1, ib * P:(ib + 1) * P],
                        rhs=wv_sb,
                        start=True, stop=True,
                    )
                ps_v = ps.rearrange("p (q s) -> p q s", s=512)[:, :, :CH]
                if g % 2 == 0:
                    nc.vector.tensor_copy(
                        out=vt_sb[:, g * 4:(g + 1) * 4, :], in_=ps_v
                    )
                else:
                    nc.scalar.copy(
                        out=vt_sb[:, g * 4:(g + 1) * 4, :], in_=ps_v
                    )

            def emit_evict(ib, jp, pp, j0, width, sums, slot, xspl):
                """Drain one [128, width] psum tile: ACT exp on cols [0, xspl),
                DVE Schraudolph on [xspl, width), partial sums to slots
                slot / slot+1."""
                nc.scalar.activation(
                    out=e_sb[:, ib, j0:j0 + xspl],
                    in_=pp[:, 0:xspl],
                    func=mybir.ActivationFunctionType.Exp,
                    bias=ebias_sb,
                    accum_out=sums[:, slot:slot + 1],
                )
                dw = width - xspl
                si = sstage.tile([P, dw], I32, tag="si",
                                 name=f"si_{ib}_{jp}_{j0}")
                nc.vector.tensor_scalar(
                    out=si, in0=pp[:, xspl:width],
                    scalar1=SCHRAU_A, scalar2=SCHRAU_B,
                    op0=mybir.AluOpType.mult,
                    op1=mybir.AluOpType.add,
                )
                nc.vector.tensor_scalar(
                    out=e_sb[:, ib, j0 + xspl:j0 + width],
                    in0=si.bitcast(F32),
                    scalar1=1.0, scalar2=0.0,
                    op0=mybir.AluOpType.mult,
                    op1=mybir.AluOpType.add,
                    accum_out=sums[:, slot + 1:slot + 2],
                )

            def emit_ib_tail(ib):
                nc.vector.reduce_sum(
                    out=r_sb[:, ib:ib + 1],
                    in_=sums_sb[:, ib, :],
                    axis=mybir.AxisListType.X,
                )
                nc.vector.reciprocal(
                    out=r_sb[:, ib:ib + 1], in_=r_sb[:, ib:ib + 1]
                )
                nc.gpsimd.tensor_scalar(
                    out=vts_sb[:, ib, :],
                    in0=vt_sb[:, ib, :],
                    scalar1=r_sb[:, ib:ib + 1],
                    scalar2=ALPHA_V,
                    op0=mybir.AluOpType.mult,
                    op1=mybir.AluOpType.mult,
                )

            with tc.tile_pool(name="mm_psum", bufs=2, space="PSUM") as mm_psum:
                # HAM warm-up: back-to-back K=128 matmuls until the first mx
                # half lands; the clock flips after one full ~3.4us window.
                wp = mm_psum.tile([P, 2048], F32, tag="ps")
                for w in range(8):
                    nc.tensor.matmul(
                        wp[:, (w % 4) * 512:(w % 4) * 512 + 512],
                        lhsT=warm_sb[:, :P],
                        rhs=warm_sb,
                        start=True, stop=True,
                    )

                # ---- Phase D (i-blocks 0..13), phase B interleaved ----
                # B's K=65 matmuls are invisible to the HAM activity monitor,
                # so they ride inside the K=128-rich phase D instead of
                # forming their own (clock-cooling) phase.
                for ib in range(NIB - 2):
                    for jp in range(2):
                        pp = mm_psum.tile([P, 2048], F32, tag="ps")
                        # dummy matmuls pad PE duty (HAM stays warm); garbage
                        # overwritten by the start=True matmuls below.
                        for d in range(3):
                            nc.tensor.matmul(
                                pp[:, (d % 4) * 512:(d % 4) * 512 + 512],
                                lhsT=warm_sb[:, :P], rhs=warm_sb,
                                start=True, stop=True,
                            )
                        for jj in range(4):
                            j0 = jp * 2048 + jj * 512
                            nc.tensor.matmul(
                                pp[:, jj * 512:(jj + 1) * 512],
                                lhsT=xda_sb[:, ib * P:(ib + 1) * P],
                                rhs=mx_sb[:, j0:j0 + 512],
                                start=True, stop=True,
                            )
                        emit_evict(ib, jp, pp, jp * 2048, 2048,
                                   sums_sb[:, ib], jp * 2, XSPL)
                    if ib < 4:
                        emit_b_group(mm_psum, ib)
                    emit_ib_tail(ib)

            # ---- last two i-blocks on a 4-bank pool + phase E ----
            # mm2 (4 banks) + ep (4 banks) coexist, so phase E's early groups
            # run while i-blocks 14/15 still compute and drain.
            with (
                tc.tile_pool(name="mm2", bufs=2, space="PSUM") as mm2,
                tc.tile_pool(name="ep", bufs=2, space="PSUM") as ep,
            ):
                for ib in (NIB - 2, NIB - 1):
                    for jq in range(4):
                        pp = mm2.tile([P, 1024], F32, tag="p2",
                                      name=f"p2_{ib}_{jq}")
                        nc.tensor.matmul(
                            pp[:, 0:512],
                            lhsT=warm_sb[:, :P], rhs=warm_sb,
                            start=True, stop=True,
                        )
                        for jj in range(2):
                            j0 = jq * 1024 + jj * 512
                            nc.tensor.matmul(
                                pp[:, jj * 512:(jj + 1) * 512],
                                lhsT=xda_sb[:, ib * P:(ib + 1) * P],
                                rhs=mx_sb[:, j0:j0 + 512],
                                start=True, stop=True,
                            )
                        emit_evict(ib, jq, pp, jq * 1024, 1024,
                                   sums2_sb[:, ib - (NIB - 2)], jq * 2, 654)
                    nc.vector.reduce_sum(
                        out=r_sb[:, ib:ib + 1],
                        in_=sums2_sb[:, ib - (NIB - 2), :],
                        axis=mybir.AxisListType.X,
                    )
                    nc.vector.reciprocal(
                        out=r_sb[:, ib:ib + 1], in_=r_sb[:, ib:ib + 1]
                    )
                    nc.gpsimd.tensor_scalar(
                        out=vts_sb[:, ib, :],
                        in0=vt_sb[:, ib, :],
                        scalar1=r_sb[:, ib:ib + 1],
                        scalar2=ALPHA_V,
                        op0=mybir.AluOpType.mult,
                        op1=mybir.AluOpType.mult,
                    )

                # ---- Phase E: out[ch, j] = sum_i vts[i, ch] e[i, j] ----
                for jc in range(8):
                    pq = ep.tile([P, KO, 512], F32, tag="po", name=f"po_{jc}")
                    for ibp in range(NPAIR):
                        for m in range(KO):
                            nc.tensor.matmul(
                                pq[:, m, :],
                                lhsT=vts_sb[:, 2 * ibp:2 * ibp + 2,
                                            m * P:(m + 1) * P],
                                rhs=e_sb[:, 2 * ibp:2 * ibp + 2,
                                         jc * 512:(jc + 1) * 512],
                                start=(ibp == 0), stop=(ibp == NPAIR - 1),
                                perf_mode=mybir.MatmulPerfMode.DoubleRow,
                            )
                    sth = stage.tile([P, KO, 512], BF16, tag="st",
                                     name=f"st_{jc}")
                    if jc % 2 == 0:
                        nc.scalar.mul(out=sth, in_=pq, mul=1.0 / ALPHA_V)
                    else:
                        nc.vector.tensor_scalar_mul(
                            out=sth, in0=pq, scalar1=1.0 / ALPHA_V
                        )
                    nc.sync.dma_start(
                        out=out_v[:, :, jc * 512:(jc + 1) * 512],
                        in_=sth,
                    )
    nc.finalize()
    return nc


_NC_CACHE = None


def _get_nc():
    global _NC_CACHE
    if _NC_CACHE is None:
        _NC_CACHE = build_bass()
    return _NC_CACHE


def make_in_maps(x_s2, x_dem, Wq, bq, Wk, bk, Wv, bv):
    scale = np.float32(CH ** -0.5)
    wk_aug = np.concatenate([Wk.T, bk[None, :]], axis=0)                 # [65, 256]
    wm = wk_aug @ (Wq * scale)                                           # [65, 256]
    wv_aug = np.concatenate([Wv.T, bv[None, :]], axis=0).astype(NP_BF16)
    ones = np.ones((1, NI), np.float32)
    B = x_s2.shape[0]
    mxs = []
    for s in range(B):
        xs = x_s2[s].reshape(CH, N)
        mx = wm @ xs                                                     # [65, 4096]
        mxb = np.zeros((P, N), np.float32)
        mxb[:DEM] = mx[:DEM]
        mxb[DEM] = mx[DEM]                # ones row carries the bk-bias
        mxs.append(np.ascontiguousarray(mxb).astype(NP_BF16))
    in_maps = []
    for c in range(NCORES):
        s, h = divmod(c, 2)
        xd = x_dem[s].reshape(DEM, N)[:, h * NI:(h + 1) * NI]
        xda = np.zeros((P, NI), np.float32)
        xda[:DEM] = xd
        xda[DEM] = 1.0
        in_maps.append({"xda": np.ascontiguousarray(xda.astype(NP_BF16)),
                        "mx": mxs[s], "wv": wv_aug})
    return in_maps


def run(inputs, trace=False, trace_cores=None):
    """Run the device kernel; returns (output, BassKernelResults)."""
    x_s2 = np.asarray(inputs["x_s2"], np.float32)
    x_dem = np.asarray(inputs["x_dem"], np.float32)
    args = {k: np.asarray(inputs[k], np.float32)
            for k in ("Wq", "bq", "Wk", "bk", "Wv", "bv")}
    in_maps = make_in_maps(x_s2, x_dem, args["Wq"], args["bq"],
                           args["Wk"], args["bk"], args["Wv"], args["bv"])
    nc = _get_nc()
    res = run_bass_kernel_spmd(nc, in_maps, core_ids=list(range(NCORES)),
                               trace=trace, trace_cores=trace_cores)
    B = x_s2.shape[0]
    out = np.empty_like(x_s2)
    for s in range(B):
        part = (res.results[2 * s]["out"].astype(np.float32)
                + res.results[2 * s + 1]["out"].astype(np.float32))
        out[s] = part.reshape(CH, 64, 64) + x_s2[s]
    return out, res


def kernel(**inputs):
    out, _ = run(inputs, trace=False)
    return out


# revision 17
# speedup vs baseline: 1.0775x; 1.0772x over previous
"""Cross-attention fusion kernel for Trainium2 (8 NeuronCores).

Reference computation (per sample b):
    q = Wq @ xs + bq            xs = x_s2[b] as [256, 4096]
    k = Wk @ xd + bk            xd = x_dem[b] as [64, 4096]
    v = Wv @ xd + bv
    attn = softmax_j(k^T q * c)             c = 256 ** -0.5
    out = v @ attn + x_s2[b]                out[ch, j] = sum_i v[ch, i] attn[i, j]

Device-side restructure (mathematically identical):
  - logits z = k^T q * c = xda^T mx with xda = [xd; 1] and
    mx = ([Wk^T; bk] @ (Wq * c)) @ xs precomputed on the host ([65, n]).
    The contraction is rank-65, so phase D runs one K=65 matmul per tile
    instead of a K=256 pair; the bk-induced per-query bias rides the ones
    row exactly and bq cancels in softmax_j -> dropped.
  - every psum tile's drain is COLUMN-SPLIT across two engines working
    concurrently (the psum pool only holds 2 tiles, so per-tile parallelism
    is the only way ACT and DVE overlap): ACT runs exp(z - ln4) via the
    activation LUT with free row-sum accumulation on the left columns; DVE
    runs Schraudolph bitcast exp (int32(A*z + B) then a bits-as-f32 cast to
    fp8 with fused accumulate) on the right columns. B encodes the same
    -ln4 shift, so the two column ranges share one scale inside each row.
  - softmax denominators folded into v columns (fp8 vts = v * r * ALPHA_V).
  - out-matmul in fp8 DoubleRow (K=256/instr), PSUM evicted as bf16 by
    ACT/DVE alternately, DMA'd per 512-column group.
  - Dummy matmuls (overwritten by the next start=True real matmul) pad the
    PE's duty cycle during the drain-bound phase D so the HAM clock gate
    holds 2.4 GHz; without them PE duty ~45% lets the clock fall to 1.2 GHz
    (the activity monitor needs a ~3.4us window of near-continuous busy).

Sharding: 8 cores = 4 samples x 2 halves of the key-pixel axis i. Each core
emits a partial out [256, 4096]; the host sums the halves and adds the
residual. No collectives.
"""

import numpy as np
import ml_dtypes

import concourse.bass as bass
import concourse.mybir as mybir
import concourse.tile as tile
from concourse import bacc
from concourse.bass_utils import run_bass_kernel_spmd

P = 128
CH = 256          # out_ch == s2_ch
DEM = 64          # dem_ch
DEM1 = DEM + 1    # + ones row (bk/bv fold)
N = 4096          # pixels per sample (j axis)
NI = 2048         # key pixels per core (i axis, half of N)
NIB = NI // P     # 16 i-blocks per core
NPAIR = NIB // 2
KO = CH // P      # 2 partition chunks of the 256-channel out axis
NCORES = 8

F32 = mybir.dt.float32
BF16 = mybir.dt.bfloat16
FP8 = mybir.dt.float8e4
I32 = mybir.dt.int32
NP_BF16 = ml_dtypes.bfloat16

# fp8 scale plan for the out-matmul (phase E): e is stored as exp(z - ln4)
# (max ~166, inside e4m3 range) and vts as v * r * ALPHA_V (O(1) values).
ALPHA_V = 8192.0
E_BIAS = -1.3862943611198906  # -ln(4)

# Schraudolph exp for the DVE-drained columns: exp(z - ln4) ~
# bitcast(int32(A*z + B)). A = 2^23/ln2; B = (127 - 2)*2^23 - C (the -2
# absorbs the -ln4 shift); C = 486500 minimizes RMS relative error (~1.8%,
# max ~3.9%, mean ~0 - scale-consistent with the ACT columns, so ACT and
# DVE columns can mix freely inside one softmax row).
SCHRAU_A = 12102203.161561485
SCHRAU_B = 125.0 * 2.0**23 - 486500.0

# Per-tile drain split: ACT handles cols [0, XSPL), DVE cols [XSPL, 2048).
# Balances (x+352)/1.2+182 on ACT against 2*(2048-x+120)/0.96 on DVE, so
# both engines finish a tile's drain together (no convoy).
XSPL = 1440


def build_bass():
    nc = bacc.Bacc(None, target_bir_lowering=False)

    xda_d = nc.dram_tensor("xda", [P, NI], BF16, kind="ExternalInput")
    mx_d = nc.dram_tensor("mx", [P, N], BF16, kind="ExternalInput")
    wv_d = nc.dram_tensor("wv", [P, CH], BF16, kind="ExternalInput")
    out_d = nc.dram_tensor("out", [CH, N], BF16, kind="ExternalOutput")

    out_v = out_d.ap().rearrange("(m p) j -> p m j", p=P)

    with tile.TileContext(nc) as tc:
        with (
            tc.tile_pool(name="consts", bufs=1) as consts,
            tc.tile_pool(name="bigs", bufs=1) as bigs,
            tc.tile_pool(name="small", bufs=1) as small,
            tc.tile_pool(name="sstage", bufs=2) as sstage,
            tc.tile_pool(name="stage", bufs=2) as stage,
        ):
            xda_sb = consts.tile([P, NI], BF16)
            nc.sync.dma_start(out=xda_sb, in_=xda_d.ap())
            wv_sb = consts.tile([P, CH], BF16)
            nc.sync.dma_start(out=wv_sb, in_=wv_d.ap())
            mx_sb = consts.tile([P, N], BF16)
            for jh in range(2):
                nc.sync.dma_start(
                    out=mx_sb[:, jh * 2048:(jh + 1) * 2048],
                    in_=mx_d.ap()[:, jh * 2048:(jh + 1) * 2048],
                )

            vt_sb = bigs.tile([P, NIB, CH], BF16)  # v^T[i, ch], i on partitions
            e_sb = bigs.tile([P, NIB, N], FP8)     # exp(z - ln4) [i, j]
            vts_sb = bigs.tile([P, NIB, CH], FP8)  # v^T * r * ALPHA_V

            r_sb = small.tile([P, NIB], F32)
            sums_sb = small.tile([P, NIB, 4], F32)
            sums2_sb = small.tile([P, 2, 8], F32)
            ebias_sb = small.tile([P, 1], F32)
            nc.vector.memset(ebias_sb, E_BIAS)
            warm_sb = small.tile([P, 512], BF16)
            nc.vector.memset(warm_sb, 0.0)

            def emit_b_group(mm_pool, g):
                ps = mm_pool.tile([P, 2048], F32, tag="ps", name=f"bps_{g}")
                for q in range(4):
                    ib = g * 4 + q
                    nc.tensor.matmul(
                        ps[:, q * 512:q * 512 + CH],
                        lhsT=xda_sb[:, ib * P:(ib + 1) * P],
                        rhs=wv_sb,
                        start=True, stop=True,
                    )
                ps_v = ps.rearrange("p (q s) -> p q s", s=512)[:, :, :CH]
                if g % 2 == 0:
                    nc.vector.tensor_copy(
                        out=vt_sb[:, g * 4:(g + 1) * 4, :], in_=ps_v
                    )
                else:
                    nc.scalar.copy(
                        out=vt_sb[:, g * 4:(g + 1) * 4, :], in_=ps_v
                    )

            def emit_evict(ib, jp, pp, j0, width, sums, slot, xspl):
                """Drain one [128, width] psum tile: ACT exp on cols [0, xspl),
                DVE Schraudolph on [xspl, width), partial sums to slots
                slot / slot+1."""
                nc.scalar.activation(
                    out=e_sb[:, ib, j0:j0 + xspl],
                    in_=pp[:, 0:xspl],
                    func=mybir.ActivationFunctionType.Exp,
                    bias=ebias_sb,
                    accum_out=sums[:, slot:slot + 1],
                )
                dw = width - xspl
                si = sstage.tile([P, dw], I32, tag="si",
                                 name=f"si_{ib}_{jp}_{j0}")
                nc.vector.tensor_scalar(
                    out=si, in0=pp[:, xspl:width],
                    scalar1=SCHRAU_A, scalar2=SCHRAU_B,
                    op0=mybir.AluOpType.mult,
                    op1=mybir.AluOpType.add,
                )
                nc.vector.tensor_scalar(
                    out=e_sb[:, ib, j0 + xspl:j0 + width],
                    in0=si.bitcast(F32),
                    scalar1=1.0, scalar2=0.0,
                    op0=mybir.AluOpType.mult,
                    op1=mybir.AluOpType.add,
                    accum_out=sums[:, slot + 1:slot + 2],
                )

            def emit_ib_tail(ib):
                nc.vector.reduce_sum(
                    out=r_sb[:, ib:ib + 1],
                    in_=sums_sb[:, ib, :],
                    axis=mybir.AxisListType.X,
                )
                nc.vector.reciprocal(
                    out=r_sb[:, ib:ib + 1], in_=r_sb[:, ib:ib + 1]
                )
                nc.gpsimd.tensor_scalar(
                    out=vts_sb[:, ib, :],
                    in0=vt_sb[:, ib, :],
                    scalar1=r_sb[:, ib:ib + 1],
                    scalar2=ALPHA_V,
                    op0=mybir.AluOpType.mult,
                    op1=mybir.AluOpType.mult,
                )

            with tc.tile_pool(name="mm_psum", bufs=2, space="PSUM") as mm_psum:
                # HAM warm-up: back-to-back K=128 matmuls until the first mx
                # half lands; the clock flips after one full ~3.4us window.
                wp = mm_psum.tile([P, 2048], F32, tag="ps")
                for w in range(6):
                    nc.tensor.matmul(
                        wp[:, (w % 4) * 512:(w % 4) * 512 + 512],
                        lhsT=warm_sb[:, :P],
                        rhs=warm_sb,
                        start=True, stop=True,
                    )

                # ---- Phase B (K=128 via zero-padded wv, so it keeps the
                # HAM activity monitor fed while the mx DMA lands) ----
                for g in range(4):
                    emit_b_group(mm_psum, g)

                # ---- Phase D (i-blocks 0..13) ----
                for ib in range(NIB - 2):
                    for jp in range(2):
                        pp = mm_psum.tile([P, 2048], F32, tag="ps")
                        # dummy matmuls pad PE duty (HAM stays warm); garbage
                        # overwritten by the start=True matmuls below.
                        for d in range(3):
                            nc.tensor.matmul(
                                pp[:, (d % 4) * 512:(d % 4) * 512 + 512],
                                lhsT=warm_sb[:, :P], rhs=warm_sb,
                                start=True, stop=True,
                            )
                        for jj in range(4):
                            j0 = jp * 2048 + jj * 512
                            nc.tensor.matmul(
                                pp[:, jj * 512:(jj + 1) * 512],
                                lhsT=xda_sb[:, ib * P:(ib + 1) * P],
                                rhs=mx_sb[:, j0:j0 + 512],
                                start=True, stop=True,
                            )
                        emit_evict(ib, jp, pp, jp * 2048, 2048,
                                   sums_sb[:, ib], jp * 2, XSPL)
                    emit_ib_tail(ib)

            # ---- last two i-blocks on a 4-bank pool + phase E ----
            # mm2 (4 banks) + ep (4 banks) coexist, so phase E's early groups
            # run while i-blocks 14/15 still compute and drain.
            with (
                tc.tile_pool(name="mm2", bufs=2, space="PSUM") as mm2,
                tc.tile_pool(name="ep", bufs=2, space="PSUM") as ep,
            ):
                for ib in (NIB - 2, NIB - 1):
                    for jq in range(4):
                        pp = mm2.tile([P, 1024], F32, tag="p2",
                                      name=f"p2_{ib}_{jq}")
                        nc.tensor.matmul(
                            pp[:, 0:512],
                            lhsT=warm_sb[:, :P], rhs=warm_sb,
                            start=True, stop=True,
                        )
                        for jj in range(2):
                            j0 = jq * 1024 + jj * 512
                            nc.tensor.matmul(
                                pp[:, jj * 512:(jj + 1) * 512],
                                lhsT=xda_sb[:, ib * P:(ib + 1) * P],
                                rhs=mx_sb[:, j0:j0 + 512],
                                start=True, stop=True,
                            )
                        emit_evict(ib, jq, pp, jq * 1024, 1024,
                                   sums2_sb[:, ib - (NIB - 2)], jq * 2, 680)
                    nc.vector.reduce_sum(
                        out=r_sb[:, ib:ib + 1],
                        in_=sums2_sb[:, ib - (NIB - 2), :],
                        axis=mybir.AxisListType.X,
                    )
                    nc.vector.reciprocal(
                        out=r_sb[:, ib:ib + 1], in_=r_sb[:, ib:ib + 1]
                    )
                    nc.gpsimd.tensor_scalar(
                        out=vts_sb[:, ib, :],
                        in0=vt_sb[:, ib, :],
                        scalar1=r_sb[:, ib:ib + 1],
                        scalar2=ALPHA_V,
                        op0=mybir.AluOpType.mult,
                        op1=mybir.AluOpType.mult,
                    )

                # ---- Phase E: out[ch, j] = sum_i vts[i, ch] e[i, j] ----
                for jc in range(8):
                    pq = ep.tile([P, KO, 512], F32, tag="po", name=f"po_{jc}")
                    for ibp in range(NPAIR):
                        for m in range(KO):
                            nc.tensor.matmul(
                                pq[:, m, :],
                                lhsT=vts_sb[:, 2 * ibp:2 * ibp + 2,
                                            m * P:(m + 1) * P],
                                rhs=e_sb[:, 2 * ibp:2 * ibp + 2,
                                         jc * 512:(jc + 1) * 512],
                                start=(ibp == 0), stop=(ibp == NPAIR - 1),
                                perf_mode=mybir.MatmulPerfMode.DoubleRow,
                            )
                    sth = stage.tile([P, KO, 512], BF16, tag="st",
                                     name=f"st_{jc}")
                    if jc % 2 == 0:
                        nc.scalar.mul(out=sth, in_=pq, mul=1.0 / ALPHA_V)
                    else:
                        nc.vector.tensor_scalar_mul(
                            out=sth, in0=pq, scalar1=1.0 / ALPHA_V
                        )
                    nc.sync.dma_start(
                        out=out_v[:, :, jc * 512:(jc + 1) * 512],
                        in_=sth,
                    )
    nc.finalize()
    return nc


_NC_CACHE = None


def _get_nc():
    global _NC_CACHE
    if _NC_CACHE is None:
        _NC_CACHE = build_bass()
    return _NC_CACHE


def make_in_maps(x_s2, x_dem, Wq, bq, Wk, bk, Wv, bv):
    scale = np.float32(CH ** -0.5)
    wk_aug = np.concatenate([Wk.T, bk[None, :]], axis=0)                 # [65, 256]
    wm = wk_aug @ (Wq * scale)                                           # [65, 256]
    wv_aug = np.zeros((P, CH), np.float32)
    wv_aug[:DEM] = Wv.T
    wv_aug[DEM] = bv
    wv_aug = wv_aug.astype(NP_BF16)
    ones = np.ones((1, NI), np.float32)
    B = x_s2.shape[0]
    mxs = []
    for s in range(B):
        xs = x_s2[s].reshape(CH, N)
        mx = wm @ xs                                                     # [65, 4096]
        mxb = np.zeros((P, N), np.float32)
        mxb[:DEM] = mx[:DEM]
        mxb[DEM] = mx[DEM]                # ones row carries the bk-bias
        mxs.append(np.ascontiguousarray(mxb).astype(NP_BF16))
    in_maps = []
    for c in range(NCORES):
        s, h = divmod(c, 2)
        xd = x_dem[s].reshape(DEM, N)[:, h * NI:(h + 1) * NI]
        xda = np.zeros((P, NI), np.float32)
        xda[:DEM] = xd
        xda[DEM] = 1.0
        in_maps.append({"xda": np.ascontiguousarray(xda.astype(NP_BF16)),
                        "mx": mxs[s], "wv": wv_aug})
    return in_maps


def run(inputs, trace=False, trace_cores=None):
    """Run the device kernel; returns (output, BassKernelResults)."""
    x_s2 = np.asarray(inputs["x_s2"], np.float32)
    x_dem = np.asarray(inputs["x_dem"], np.float32)
    args = {k: np.asarray(inputs[k], np.float32)
            for k in ("Wq", "bq", "Wk", "bk", "Wv", "bv")}
    in_maps = make_in_maps(x_s2, x_dem, args["Wq"], args["bq"],
                           args["Wk"], args["bk"], args["Wv"], args["bv"])
    nc = _get_nc()
    res = run_bass_kernel_spmd(nc, in_maps, core_ids=list(range(NCORES)),
                               trace=trace, trace_cores=trace_cores)
    B = x_s2.shape[0]
    out = np.empty_like(x_s2)
    for s in range(B):
        part = (res.results[2 * s]["out"].astype(np.float32)
                + res.results[2 * s + 1]["out"].astype(np.float32))
        out[s] = part.reshape(CH, 64, 64) + x_s2[s]
    return out, res


def kernel(**inputs):
    out, _ = run(inputs, trace=False)
    return out
